# revision 20
# baseline (speedup 1.0000x reference)
"""Trainium2 Bass kernel for the gated-MLP-over-ring-buffer problem.

Reference computation (B=512, M=128, V=256, H=256, IN = M*V = 32768):
    mem    = roll(memory, 1, axis=1); mem[:, 0, :] = x        # [B, M, V]
    flat   = mem.reshape(B, IN)                                # [B, 32768]
    h      = tanh(flat @ W1 + b1) * sigmoid(flat @ Wg + bg)    # [B, 256]
    logits = h @ W2 + b2                                       # [B, 256]

Strategy (8 NeuronCores, one trn2 chip):
  - Contraction-shard the two big GEMMs: core c owns k-rows
    [4096c, 4096(c+1)) of W1/Wg and the matching slab of flat.T
    (host-prepared, transposed + bf16 so SBUF tiles load at line rate,
    partition-major so each k-group is one contiguous DMA).
  - Each core computes partial P1.T / Pg.T = W.T @ flat.T  -> [H, B]
    accumulated over its 32 k-chunks in PSUM (bf16 operands, f32 acc).
  - Cross-core reduction of the [2H, B] partials in bf16, scattered
    over B (AllToAll) so core c ends up with batch cols [64c, 64c+64).
    Split in two K-halves so the first AllToAll hides under the second
    half of compute; a dummy AllToAll at kernel start absorbs the
    one-time collective barrier / control-plane warmup.
  - Each core applies bias + tanh/sigmoid gating and the small W2
    GEMM (bf16) for its batch chunk, writing logits.T [V, 64].
  - Host assembles/transposes the 8 chunks back to [B, V].
"""

import numpy as np

import concourse.bacc as bacc
import concourse.bass as bass
import concourse.mybir as mybir
import concourse.tile as tile
from concourse import bass_utils

B, M, V, H = 512, 128, 256, 256
IN = M * V              # 32768
NCORES = 8
KC = IN // NCORES       # 4096 contraction rows per core
NKG = 8                 # DMA k-groups per core
KB_PER_G = KC // (NKG * 128)  # 4 k-chunks of 128 per group
BCHUNK = B // NCORES    # 64 batch columns per core after reduce-scatter
SPLIT_G = 4             # k-groups in the first (early-flushed) half
WARMUP_MM = 12

F32 = mybir.dt.float32
BF16 = mybir.dt.bfloat16
AF = mybir.ActivationFunctionType
RG = [list(range(NCORES))]

_CACHE = {}


def _build():
    nc = bacc.Bacc(
        "TRN2",
        target_bir_lowering=False,
        debug=False,
        enable_asserts=False,
        num_devices=NCORES,
    )

    # Per-core external inputs (host-packed, partition-major, bf16).
    memT = nc.dram_tensor("memT", [NKG, 128, KB_PER_G, B], BF16, kind="ExternalInput")
    wpk = nc.dram_tensor("wpk", [NKG, 128, KB_PER_G, 2, H], BF16, kind="ExternalInput")
    w2pk = nc.dram_tensor("w2pk", [128, 2, V], BF16, kind="ExternalInput")
    # packed biases: cols = [b1_lo, b1_hi, bg_lo, bg_hi, b2_lo, b2_hi]
    bpk = nc.dram_tensor("bpk", [128, 6], F32, kind="ExternalInput")
    outT = nc.dram_tensor("outT", [V, BCHUNK], F32, kind="ExternalOutput")

    with tile.TileContext(nc) as tc:
        with (
            tc.tile_pool(name="xg", bufs=NKG) as xpool,
            tc.tile_pool(name="wt", bufs=NKG) as wpool,
            tc.tile_pool(name="part", bufs=1) as ppool,
            tc.tile_pool(name="s2", bufs=1) as s2pool,
            tc.tile_pool(name="psum1", bufs=1, space="PSUM") as psum1,
            tc.tile_pool(name="dram", bufs=1, space="DRAM") as dpool,
        ):
            # Pre-warm the PE HAM clock gate with dummy matmuls while the
            # first input DMAs are in flight; wsrc memset leads the gpsimd
            # queue so the warmup starts right after the engine preamble.
            wsrc = s2pool.tile([128, B], BF16, tag="wsrc")
            nc.gpsimd.memset(wsrc[:], 0.0)
            wps = psum1.tile([128, B], F32, tag="acc7", name="wps")
            for i in range(WARMUP_MM):
                nc.tensor.matmul(
                    wps[:],
                    wsrc[:, 0:128],
                    wsrc[:],
                    start=(i == 0),
                    stop=(i == WARMUP_MM - 1),
                )

            # ---- tiny early collective: the CC stream's first op after the
            # rank barrier pays a cold-start cost (~8-10us); burning it on a
            # 1KiB dummy during the DMA/compute phase makes the real
            # AllToAll run warm.
            dumin = dpool.tile([NCORES, 64], BF16, tag="dumin", name="dumin")
            dumout = dpool.tile([NCORES, 64], BF16, tag="dumout", name="dumout")
            dum = s2pool.tile([NCORES, 64], BF16, tag="dum")
            nc.gpsimd.memset(dum[:], 0.0)
            nc.sync.dma_start(out=dumin[:], in_=dum[:])
            nc.gpsimd.collective_compute(
                "AllToAll",
                mybir.AluOpType.bypass,
                replica_groups=RG,
                ins=[dumin[:].opt()],
                outs=[dumout[:].opt()],
            )

            # Stage-2 constants on the (otherwise idle) gpsimd SWDGE queue.
            bt = s2pool.tile([128, 6], F32, tag="bias")
            nc.gpsimd.dma_start(out=bt[:], in_=bpk[:, :])
            w2t = s2pool.tile([128, 2, V], BF16, tag="w2")
            nc.gpsimd.dma_start(out=w2t[:], in_=w2pk[:, :, :])

            # ---------------- stage 1: partial W.T @ flat.T ----------------
            acc = [
                psum1.tile([128, B], F32, tag=f"acc{i}", name=f"acc_{i}")
                for i in range(4)
            ]
            ccin = dpool.tile(
                [NCORES, 128, 4, BCHUNK], BF16, tag="ccin", name="ccin"
            )
            ccout = dpool.tile(
                [NCORES, 128, 4, BCHUNK], BF16, tag="ccout", name="ccout"
            )

            NK = NKG * KB_PER_G
            for kg in range(NKG):
                xg = xpool.tile([128, KB_PER_G, B], BF16, tag="xg")
                wt = wpool.tile([128, KB_PER_G, 2, H], BF16, tag="wt")
                if kg == 0:
                    # Piecewise first tile so the first matmuls start as soon
                    # as the first 128-k chunk lands, not the whole 1 MiB.
                    for kb in range(KB_PER_G):
                        nc.sync.dma_start(
                            out=xg[:, kb : kb + 1, :],
                            in_=memT[kg][:, kb : kb + 1, :],
                        )
                        nc.scalar.dma_start(
                            out=wt[:, kb : kb + 1], in_=wpk[kg][:, kb : kb + 1]
                        )
                else:
                    nc.sync.dma_start(out=xg[:], in_=memT[kg])
                    nc.scalar.dma_start(out=wt[:], in_=wpk[kg])
                for kb in range(KB_PER_G):
                    k = kg * KB_PER_G + kb
                    rhs = xg[:, kb, :]
                    for w in range(2):
                        for h in range(2):
                            nc.tensor.matmul(
                                acc[2 * w + h][:],
                                wt[:, kb, w, bass.ts(h, 128)],
                                rhs,
                                start=(k == 0),
                                stop=(k == NK - 1),
                            )
                if kg == 1:
                    # Pre-warm the Tanh/Sigmoid activation tables while the
                    # scalar engine is otherwise idle (first use of each
                    # table pays a ~1.3us load).
                    warm = s2pool.tile([128, 1], F32, tag="warm")
                    nc.gpsimd.memset(warm[:], 0.0)
                    warm2 = s2pool.tile([128, 1], F32, tag="warm2")
                    nc.scalar.activation(warm2[:], warm[:], AF.Tanh)
                    nc.scalar.activation(warm[:], warm2[:], AF.Sigmoid)

            # PSUM -> SBUF (cast bf16, c-major) -> DRAM -> single AllToAll
            sb = ppool.tile([128, NCORES, 4, BCHUNK], BF16, tag="sb", name="sb")
            for t in range(4):
                nc.vector.tensor_copy(
                    sb[:, :, t, :],
                    acc[t][:].rearrange("p (c b) -> p c b", c=NCORES),
                )
            # One store: per-piece HWDGE issue cost (~0.6us each) plus the
            # multi-sem consolidation before the trigger costs more than the
            # single 512KiB drain.
            nc.sync.dma_start(
                out=ccin[:].rearrange("c p t b -> p c (t b)"),
                in_=sb[:].rearrange("p c t b -> p c (t b)"),
            )
            nc.gpsimd.collective_compute(
                "AllToAll",
                mybir.AluOpType.bypass,
                replica_groups=RG,
                ins=[ccin[:].opt()],
                outs=[ccout[:].opt()],
            )

            # ------------- local reduction of received slabs -------------
            # Two parallel DMAs (sync + scalar HWDGE queues) halve the drain.
            rr = s2pool.tile([128, NCORES, 4, BCHUNK], BF16, tag="rr", name="rr")
            nc.scalar.dma_start(
                out=rr[:, 0:4].rearrange("p c t b -> p c (t b)"),
                in_=ccout[0:4].rearrange("c p t b -> p c (t b)"),
            )
            nc.sync.dma_start(
                out=rr[:, 4:8].rearrange("p c t b -> p c (t b)"),
                in_=ccout[4:8].rearrange("c p t b -> p c (t b)"),
            )
            t1 = s2pool.tile([128, 4, 4, BCHUNK], BF16, tag="t1", name="t1")
            nc.vector.tensor_add(t1[:], rr[:, 0:4, :, :], rr[:, 4:8, :, :])
            t2 = s2pool.tile([128, 2, 4, BCHUNK], BF16, tag="t2", name="t2")
            nc.vector.tensor_add(t2[:], t1[:, 0:2, :, :], t1[:, 2:4, :, :])
            s2 = s2pool.tile([128, 4, BCHUNK], F32, tag="s2in")
            nc.vector.tensor_add(s2[:], t2[:, 0, :, :], t2[:, 1, :, :])

            # ---------------- stage 2: gate + W2 ----------------
            hT = []
            for i in range(2):
                th = s2pool.tile([128, BCHUNK], BF16, tag=f"th{i}", name=f"th{i}")
                nc.scalar.activation(th[:], s2[:, i, :], AF.Tanh, bias=bt[:, i : i + 1])
                sg = s2pool.tile([128, BCHUNK], BF16, tag=f"sg{i}", name=f"sg{i}")
                nc.scalar.activation(
                    sg[:], s2[:, 2 + i, :], AF.Sigmoid, bias=bt[:, 2 + i : 3 + i]
                )
                ht = s2pool.tile([128, BCHUNK], BF16, tag=f"ht{i}", name=f"ht{i}")
                nc.vector.tensor_mul(ht[:], th[:], sg[:])
                hT.append(ht)

            for v in range(2):
                ps = psum1.tile([128, BCHUNK], F32, tag=f"acc{v}", name=f"ps2_{v}")
                for i in range(2):
                    nc.tensor.matmul(
                        ps[:],
                        w2t[:, i, bass.ts(v, 128)],
                        hT[i][:],
                        start=(i == 0),
                        stop=(i == 1),
                    )
                ot = s2pool.tile([128, BCHUNK], F32, tag=f"ot{v}", name=f"ot{v}")
                nc.vector.tensor_scalar_add(ot[:], ps[:], bt[:, 4 + v : 5 + v])
                eng = nc.sync if v == 0 else nc.scalar
                eng.dma_start(out=outT[bass.ts(v, 128), :], in_=ot[:])

    nc.compile()
    return nc


def _shard(x, memory, W1, b1, Wg, bg, W2, b2):
    """Build the 8 per-core input maps from the full problem inputs."""
    import ml_dtypes

    bf16 = ml_dtypes.bfloat16
    x = np.asarray(x, dtype=np.float32)
    memory = np.asarray(memory, dtype=np.float32)
    W1 = np.asarray(W1, dtype=np.float32)
    Wg = np.asarray(Wg, dtype=np.float32)
    W2 = np.asarray(W2, dtype=np.float32)
    b1 = np.asarray(b1, dtype=np.float32)
    bg = np.asarray(bg, dtype=np.float32)
    b2 = np.asarray(b2, dtype=np.float32)

    # rolled ring buffer, flattened and transposed: [IN, B]
    flatT = np.empty((IN, B), dtype=bf16)
    flatT[:V] = x.T
    flatT[V:] = memory[:, : M - 1, :].reshape(B, IN - V).T
    bpk = np.ascontiguousarray(
        np.stack([b1[:128], b1[128:], bg[:128], bg[128:], b2[:128], b2[128:]], axis=1)
    )
    w2pk = np.ascontiguousarray(
        W2.reshape(2, 128, V).transpose(1, 0, 2).astype(bf16)
    )

    in_maps = []
    for c in range(NCORES):
        sl = slice(KC * c, KC * (c + 1))
        # [NKG, KB, 128, B] -> partition-major [NKG, 128, KB, B]
        mT = np.ascontiguousarray(
            flatT[sl].reshape(NKG, KB_PER_G, 128, B).transpose(0, 2, 1, 3)
        )
        w1s = W1[sl].reshape(NKG, KB_PER_G, 128, H)
        wgs = Wg[sl].reshape(NKG, KB_PER_G, 128, H)
        wpk = np.ascontiguousarray(
            np.stack([w1s, wgs], axis=3).transpose(0, 2, 1, 3, 4).astype(bf16)
        )
        in_maps.append(
            {"memT": mT, "wpk": wpk, "w2pk": w2pk, "bpk": bpk}
        )
    return in_maps


def _get_nc():
    if "nc" not in _CACHE:
        _CACHE["nc"] = _build()
    return _CACHE["nc"]


def kernel(x, memory, W1, b1, Wg, bg, W2, b2, **run_kwargs):
    nc = _get_nc()
    in_maps = _shard(x, memory, W1, b1, Wg, bg, W2, b2)
    res = bass_utils.run_bass_kernel_spmd(
        nc, in_maps, core_ids=list(range(NCORES)), **run_kwargs
    )
    _CACHE["last_results"] = res
    out = np.empty((B, V), dtype=np.float32)
    for c in range(NCORES):
        out[c * BCHUNK : (c + 1) * BCHUNK, :] = res.results[c]["outT"].T
    return out


# revision 22
# speedup vs baseline: 1.0309x; 1.0309x over previous
"""Trainium2 Bass kernel for the gated-MLP-over-ring-buffer problem.

Reference computation (B=512, M=128, V=256, H=256, IN = M*V = 32768):
    mem    = roll(memory, 1, axis=1); mem[:, 0, :] = x        # [B, M, V]
    flat   = mem.reshape(B, IN)                                # [B, 32768]
    h      = tanh(flat @ W1 + b1) * sigmoid(flat @ Wg + bg)    # [B, 256]
    logits = h @ W2 + b2                                       # [B, 256]

Strategy (8 NeuronCores, one trn2 chip):
  - Contraction-shard the two big GEMMs: core c owns k-rows
    [4096c, 4096(c+1)) of W1/Wg and the matching slab of flat.T
    (host-prepared, transposed + bf16 so SBUF tiles load at line rate,
    partition-major so each k-group is one contiguous DMA).
  - Each core computes partial P1.T / Pg.T = W.T @ flat.T  -> [H, B]
    accumulated over its 32 k-chunks in PSUM (bf16 operands, f32 acc).
  - Cross-core reduction of the [2H, B] partials in bf16, scattered
    over B (AllToAll) so core c ends up with batch cols [64c, 64c+64).
    Split in two K-halves so the first AllToAll hides under the second
    half of compute; a dummy AllToAll at kernel start absorbs the
    one-time collective barrier / control-plane warmup.
  - Each core applies bias + tanh/sigmoid gating and the small W2
    GEMM (bf16) for its batch chunk, writing logits.T [V, 64].
  - Host assembles/transposes the 8 chunks back to [B, V].
"""

import numpy as np

import concourse.bacc as bacc
import concourse.bass as bass
import concourse.mybir as mybir
import concourse.tile as tile
from concourse import bass_utils

B, M, V, H = 512, 128, 256, 256
IN = M * V              # 32768
NCORES = 8
KC = IN // NCORES       # 4096 contraction rows per core
NKG = 8                 # DMA k-groups per core
KB_PER_G = KC // (NKG * 128)  # 4 k-chunks of 128 per group
BCHUNK = B // NCORES    # 64 batch columns per core after reduce-scatter
SPLIT_G = 4             # k-groups in the first (early-flushed) half
WARMUP_MM = 8

F32 = mybir.dt.float32
BF16 = mybir.dt.bfloat16
AF = mybir.ActivationFunctionType
RG = [list(range(NCORES))]

_CACHE = {}


def _build():
    nc = bacc.Bacc(
        "TRN2",
        target_bir_lowering=False,
        debug=False,
        enable_asserts=False,
        num_devices=NCORES,
    )

    # Per-core external inputs (host-packed, partition-major, bf16).
    memT = nc.dram_tensor("memT", [NKG, 128, KB_PER_G, B], BF16, kind="ExternalInput")
    wpk = nc.dram_tensor("wpk", [NKG, 128, KB_PER_G, 2, H], BF16, kind="ExternalInput")
    w2pk = nc.dram_tensor("w2pk", [128, 2, V], BF16, kind="ExternalInput")
    # packed biases: cols = [b1_lo, b1_hi, bg_lo, bg_hi, b2_lo, b2_hi]
    bpk = nc.dram_tensor("bpk", [128, 6], F32, kind="ExternalInput")
    outT = nc.dram_tensor("outT", [V, BCHUNK], F32, kind="ExternalOutput")

    with tile.TileContext(nc) as tc:
        with (
            tc.tile_pool(name="xg", bufs=NKG) as xpool,
            tc.tile_pool(name="wt", bufs=NKG) as wpool,
            tc.tile_pool(name="part", bufs=1) as ppool,
            tc.tile_pool(name="s2", bufs=1) as s2pool,
            tc.tile_pool(name="psum1", bufs=1, space="PSUM") as psum1,
            tc.tile_pool(name="dram", bufs=1, space="DRAM") as dpool,
        ):
            # Pre-warm the PE HAM clock gate with dummy matmuls while the
            # first input DMAs are in flight; wsrc memset leads the gpsimd
            # queue so the warmup starts right after the engine preamble.
            wsrc = s2pool.tile([128, B], BF16, tag="wsrc")
            nc.gpsimd.memset(wsrc[:], 0.0)
            wps = psum1.tile([128, B], F32, tag="acc7", name="wps")
            for i in range(WARMUP_MM):
                nc.tensor.matmul(
                    wps[:],
                    wsrc[:, 0:128],
                    wsrc[:],
                    start=(i == 0),
                    stop=(i == WARMUP_MM - 1),
                )

            # ---- tiny early collective: the CC stream's first op after the
            # rank barrier pays a cold-start cost (~8-10us); burning it on a
            # 1KiB dummy during the DMA/compute phase makes the real
            # AllToAll run warm.
            dumin = dpool.tile([NCORES, 64], BF16, tag="dumin", name="dumin")
            dumout = dpool.tile([NCORES, 64], BF16, tag="dumout", name="dumout")
            dum = s2pool.tile([NCORES, 64], BF16, tag="dum")
            nc.gpsimd.memset(dum[:], 0.0)
            nc.sync.dma_start(out=dumin[:], in_=dum[:])
            nc.gpsimd.collective_compute(
                "AllToAll",
                mybir.AluOpType.bypass,
                replica_groups=RG,
                ins=[dumin[:].opt()],
                outs=[dumout[:].opt()],
            )

            # Stage-2 constants on the (otherwise idle) gpsimd SWDGE queue.
            bt = s2pool.tile([128, 6], F32, tag="bias")
            nc.gpsimd.dma_start(out=bt[:], in_=bpk[:, :])
            w2t = s2pool.tile([128, 2, V], BF16, tag="w2")
            nc.gpsimd.dma_start(out=w2t[:], in_=w2pk[:, :, :])

            # ---------------- stage 1: partial W.T @ flat.T ----------------
            acc = [
                psum1.tile([128, B], F32, tag=f"acc{i}", name=f"acc_{i}")
                for i in range(4)
            ]
            ccin = dpool.tile(
                [NCORES, 128, 4, BCHUNK], BF16, tag="ccin", name="ccin"
            )
            ccout = dpool.tile(
                [NCORES, 128, 4, BCHUNK], BF16, tag="ccout", name="ccout"
            )

            NK = NKG * KB_PER_G
            for kg in range(NKG):
                xg = xpool.tile([128, KB_PER_G, B], BF16, tag="xg")
                wt = wpool.tile([128, KB_PER_G, 2, H], BF16, tag="wt")
                if kg == 0:
                    # Piecewise first tile so the first matmuls start as soon
                    # as the first 128-k chunk lands, not the whole 1 MiB.
                    for kb in range(KB_PER_G):
                        nc.sync.dma_start(
                            out=xg[:, kb : kb + 1, :],
                            in_=memT[kg][:, kb : kb + 1, :],
                        )
                        nc.scalar.dma_start(
                            out=wt[:, kb : kb + 1], in_=wpk[kg][:, kb : kb + 1]
                        )
                else:
                    nc.sync.dma_start(out=xg[:], in_=memT[kg])
                    nc.scalar.dma_start(out=wt[:], in_=wpk[kg])
                for kb in range(KB_PER_G):
                    k = kg * KB_PER_G + kb
                    rhs = xg[:, kb, :]
                    for w in range(2):
                        for h in range(2):
                            nc.tensor.matmul(
                                acc[2 * w + h][:],
                                wt[:, kb, w, bass.ts(h, 128)],
                                rhs,
                                start=(k == 0),
                                stop=(k == NK - 1),
                            )
                if kg == 1:
                    # Pre-warm the Tanh/Sigmoid activation tables while the
                    # scalar engine is otherwise idle (first use of each
                    # table pays a ~1.3us load).
                    warm = s2pool.tile([128, 1], F32, tag="warm")
                    nc.gpsimd.memset(warm[:], 0.0)
                    warm2 = s2pool.tile([128, 1], F32, tag="warm2")
                    nc.scalar.activation(warm2[:], warm[:], AF.Tanh)
                    nc.scalar.activation(warm[:], warm2[:], AF.Sigmoid)

            # PSUM -> SBUF (cast bf16, c-major) -> DRAM -> single AllToAll
            sb = ppool.tile([128, NCORES, 4, BCHUNK], BF16, tag="sb", name="sb")
            for t in range(4):
                nc.vector.tensor_copy(
                    sb[:, :, t, :],
                    acc[t][:].rearrange("p (c b) -> p c b", c=NCORES),
                )
            # One store: per-piece HWDGE issue cost (~0.6us each) plus the
            # multi-sem consolidation before the trigger costs more than the
            # single 512KiB drain.
            nc.sync.dma_start(
                out=ccin[:].rearrange("c p t b -> p c (t b)"),
                in_=sb[:].rearrange("p c t b -> p c (t b)"),
            )
            nc.gpsimd.collective_compute(
                "AllToAll",
                mybir.AluOpType.bypass,
                replica_groups=RG,
                ins=[ccin[:].opt()],
                outs=[ccout[:].opt()],
            )

            # ------------- local reduction of received slabs -------------
            # Three parallel DMAs (sync + scalar HWDGE, gpsimd SWDGE) split
            # the post-collective read: it measured ~4.7us on two queues.
            rr = s2pool.tile([128, NCORES, 4, BCHUNK], BF16, tag="rr", name="rr")
            nc.scalar.dma_start(
                out=rr[:, 0:3].rearrange("p c t b -> p c (t b)"),
                in_=ccout[0:3].rearrange("c p t b -> p c (t b)"),
            )
            nc.sync.dma_start(
                out=rr[:, 3:6].rearrange("p c t b -> p c (t b)"),
                in_=ccout[3:6].rearrange("c p t b -> p c (t b)"),
            )
            nc.gpsimd.dma_start(
                out=rr[:, 6:8].rearrange("p c t b -> p c (t b)"),
                in_=ccout[6:8].rearrange("c p t b -> p c (t b)"),
            )
            t1 = s2pool.tile([128, 4, 4, BCHUNK], BF16, tag="t1", name="t1")
            nc.vector.tensor_add(t1[:], rr[:, 0:4, :, :], rr[:, 4:8, :, :])
            t2 = s2pool.tile([128, 2, 4, BCHUNK], BF16, tag="t2", name="t2")
            nc.vector.tensor_add(t2[:], t1[:, 0:2, :, :], t1[:, 2:4, :, :])
            s2 = s2pool.tile([128, 4, BCHUNK], F32, tag="s2in")
            nc.vector.tensor_add(s2[:], t2[:, 0, :, :], t2[:, 1, :, :])

            # ---------------- stage 2: gate + W2 ----------------
            hT = []
            for i in range(2):
                th = s2pool.tile([128, BCHUNK], BF16, tag=f"th{i}", name=f"th{i}")
                nc.scalar.activation(th[:], s2[:, i, :], AF.Tanh, bias=bt[:, i : i + 1])
                sg = s2pool.tile([128, BCHUNK], BF16, tag=f"sg{i}", name=f"sg{i}")
                nc.scalar.activation(
                    sg[:], s2[:, 2 + i, :], AF.Sigmoid, bias=bt[:, 2 + i : 3 + i]
                )
                ht = s2pool.tile([128, BCHUNK], BF16, tag=f"ht{i}", name=f"ht{i}")
                nc.vector.tensor_mul(ht[:], th[:], sg[:])
                hT.append(ht)

            for v in range(2):
                ps = psum1.tile([128, BCHUNK], F32, tag=f"acc{v}", name=f"ps2_{v}")
                for i in range(2):
                    nc.tensor.matmul(
                        ps[:],
                        w2t[:, i, bass.ts(v, 128)],
                        hT[i][:],
                        start=(i == 0),
                        stop=(i == 1),
                    )
                ot = s2pool.tile([128, BCHUNK], F32, tag=f"ot{v}", name=f"ot{v}")
                nc.vector.tensor_scalar_add(ot[:], ps[:], bt[:, 4 + v : 5 + v])
                eng = nc.sync if v == 0 else nc.scalar
                eng.dma_start(out=outT[bass.ts(v, 128), :], in_=ot[:])

    nc.compile()
    return nc


def _shard(x, memory, W1, b1, Wg, bg, W2, b2):
    """Build the 8 per-core input maps from the full problem inputs."""
    import ml_dtypes

    bf16 = ml_dtypes.bfloat16
    x = np.asarray(x, dtype=np.float32)
    memory = np.asarray(memory, dtype=np.float32)
    W1 = np.asarray(W1, dtype=np.float32)
    Wg = np.asarray(Wg, dtype=np.float32)
    W2 = np.asarray(W2, dtype=np.float32)
    b1 = np.asarray(b1, dtype=np.float32)
    bg = np.asarray(bg, dtype=np.float32)
    b2 = np.asarray(b2, dtype=np.float32)

    # rolled ring buffer, flattened and transposed: [IN, B]
    flatT = np.empty((IN, B), dtype=bf16)
    flatT[:V] = x.T
    flatT[V:] = memory[:, : M - 1, :].reshape(B, IN - V).T
    bpk = np.ascontiguousarray(
        np.stack([b1[:128], b1[128:], bg[:128], bg[128:], b2[:128], b2[128:]], axis=1)
    )
    w2pk = np.ascontiguousarray(
        W2.reshape(2, 128, V).transpose(1, 0, 2).astype(bf16)
    )

    in_maps = []
    for c in range(NCORES):
        sl = slice(KC * c, KC * (c + 1))
        # [NKG, KB, 128, B] -> partition-major [NKG, 128, KB, B]
        mT = np.ascontiguousarray(
            flatT[sl].reshape(NKG, KB_PER_G, 128, B).transpose(0, 2, 1, 3)
        )
        w1s = W1[sl].reshape(NKG, KB_PER_G, 128, H)
        wgs = Wg[sl].reshape(NKG, KB_PER_G, 128, H)
        wpk = np.ascontiguousarray(
            np.stack([w1s, wgs], axis=3).transpose(0, 2, 1, 3, 4).astype(bf16)
        )
        in_maps.append(
            {"memT": mT, "wpk": wpk, "w2pk": w2pk, "bpk": bpk}
        )
    return in_maps


def _get_nc():
    if "nc" not in _CACHE:
        _CACHE["nc"] = _build()
    return _CACHE["nc"]


def kernel(x, memory, W1, b1, Wg, bg, W2, b2, **run_kwargs):
    nc = _get_nc()
    in_maps = _shard(x, memory, W1, b1, Wg, bg, W2, b2)
    res = bass_utils.run_bass_kernel_spmd(
        nc, in_maps, core_ids=list(range(NCORES)), **run_kwargs
    )
    _CACHE["last_results"] = res
    out = np.empty((B, V), dtype=np.float32)
    for c in range(NCORES):
        out[c * BCHUNK : (c + 1) * BCHUNK, :] = res.results[c]["outT"].T
    return out
